# revision 24
# baseline (speedup 1.0000x reference)
"""Trainium2 Bass kernel for causal multi-head attention block (B=8, S=1024, D=1024, H=16).

Sharding: pure batch data-parallelism - one batch element per NeuronCore (B=8, 8 cores).
Each core runs the full transformer block on its [S, D] slice; no collectives.

Per-core algorithm (layouts chosen so no on-device transposes are needed):
  - Host passes x^T and all W^T pre-strided into the SBUF partition layout
    [p, db, cols] so every big DMA is 128 large contiguous descriptors.
  - QKV projections run in fp8e4 DoubleRow mode (two 128-deep k-tiles per pass,
    so K=256 per matmul at bf16-rate): host scales x by 8 and Wq/Wk/Wv by 256
    (keeps U(-1/32,1/32) weights out of fp8 denormals); the PSUM drain rescales
    by 1/2048 and adds the bias.
  - Q^T, K^T land as [o, s] bf16; per-head [dk, S] slices feed the scores matmul.
  - V lands natural [s, o] in fp8 per head with an appended ones column
    (V' = [V | padmask]) so the ctx matmul also yields the softmax denominator
    s0 in PSUM row 64.  Padded keys have V rows zeroed.
  - scoresT[k, q] = matmul(lhsT=K^T head slice, rhs=Q^T head slice) in bf16
    (contraction dk=64; even/odd heads at PE row groups 0/64 run concurrently).
    k-blocks processed in pairs sharing one 2-bank PSUM tile.
  - Softmax without max-subtraction: u = exp(0.125*scores) on ACT, written as
    fp8.  Causal masking multiplies u by 0/1 masks on DVE: a [128,128] triangle
    for the pair's lower block, and a [128,256] [zeros|triangle] extended mask
    for the upper block (which shares the lower block's column window, so its
    below-diagonal garbage must be zeroed for the DoubleRow ctx matmul).
  - ctx matmul in fp8 DoubleRow over k-block pairs (K=256 per pass):
    lhsT=V' pair, rhs=u pair.
  - Rows with a fully-masked causal window (s0 == 0) must match the reference's
    softmax(-1e9*ones) = uniform over ALL 1024 keys: ctx = (ctx_u + bad*sumV)/
    (s0 + 1024*bad), bad = (s0 <= 1e-30), sumV = column sums of V'.
  - The per-(head,q) normalizers are broadcast across partitions with a selector
    matmul (lhsT = 0/1 pair-selector, rhs = the 32-row table) into PSUM.
  - Out projection in bf16: out[s, o] = matmul(lhsT=ctx^T block, rhs=Wo^T);
    residual x+bo added, LayerNorm via bn_stats/bn_aggr.  gamma/beta applied on
    host (exact no-op for the reference's ones/zeros).
"""

import sys

import numpy as np

if "/opt/trn_rl_repo" not in sys.path:
    sys.path.insert(0, "/opt/trn_rl_repo")

S = 1024
D = 1024
H = 16
DK = 64
P = 128
DB = D // P  # 8 d-blocks
SB = S // P  # 8 s-blocks
NEG = -1.0e9
SCALE = 0.125  # 1/sqrt(64)
EPS = 1e-5
N_CORES = 8
XS = 8.0  # host scale on x for fp8
WS = 256.0  # host scale on Wq/Wk/Wv for fp8
INV_XW = 1.0 / (XS * WS)

_built = None


def _kbs(qc):
    """k-block pairs needed for q-chunk qc (q in [qc*512, qc*512+512))."""
    return [(0, 1), (2, 3)] if qc == 0 else [(0, 1), (2, 3), (4, 5), (6, 7)]


def _vs(kb, qc):
    """first causal q column within the 512-wide chunk for k-block kb."""
    return max(0, kb * P - qc * 512)


def _build():
    import concourse.mybir as mybir
    import concourse.tile as tile
    from concourse import bacc

    f32 = mybir.dt.float32
    bf16 = mybir.dt.bfloat16
    fp8 = mybir.dt.float8e4
    Alu = mybir.AluOpType
    Act = mybir.ActivationFunctionType
    DR = mybir.MatmulPerfMode.DoubleRow

    nc = bacc.Bacc()

    # ---- DRAM I/O (pre-strided [p, db, cols] contiguous layouts from host) ----
    xt_lo_d = nc.dram_tensor("xt_lo", [P, 4, S], fp8, kind="ExternalInput")
    xt_hi_d = nc.dram_tensor("xt_hi", [P, 4, S], fp8, kind="ExternalInput")
    xr_d = nc.dram_tensor("xr", [S, D], f32, kind="ExternalInput")  # x + bo
    w_d = {}
    for wname in ("wq", "wk", "wv", "wo"):
        for oc in range(2):
            n = f"{wname}{oc}"
            w_d[n] = nc.dram_tensor(n, [P, DB, 512], fp8, kind="ExternalInput")
    bqs_d = nc.dram_tensor("bqs", [P, DB], f32, kind="ExternalInput")
    bks_d = nc.dram_tensor("bks", [P, DB], f32, kind="ExternalInput")
    padm_d = nc.dram_tensor("padm", [P, SB], f32, kind="ExternalInput")  # 1 valid / 0 pad
    sumv_d = nc.dram_tensor("sumv", [P, DB], f32, kind="ExternalInput")
    pairsel_d = nc.dram_tensor("pairsel", [2 * H, SB * P], bf16, kind="ExternalInput")
    causal_d = nc.dram_tensor("causal", [P, P], fp8, kind="ExternalInput")  # 0/1
    causx_d = nc.dram_tensor("causx", [P, 2 * P], fp8, kind="ExternalInput")  # [0|tri]
    out_d = nc.dram_tensor("out", [S, D], f32, kind="ExternalOutput")

    with tile.TileContext(nc) as tc:
        with (
            tc.tile_pool(name="singles", bufs=1) as singles,
            tc.tile_pool(name="qt", bufs=1) as qt_pool,
            tc.tile_pool(name="kt", bufs=1) as kt_pool,
            tc.tile_pool(name="vp", bufs=1) as vp_pool,
            tc.tile_pool(name="xtp", bufs=2) as xtp,  # two halves of x^T (fp8)
            tc.tile_pool(name="wp", bufs=3) as wp,  # fp8 qkv weight chunks
            tc.tile_pool(name="up", bufs=4) as up,  # fp8 u chunks (2 live/iter)
            tc.tile_pool(name="wop", bufs=2) as wop,  # fp8 wo chunks
            tc.tile_pool(name="qt8", bufs=1) as qt8_pool,  # fp8 normalized ctx
            tc.tile_pool(name="xres", bufs=2) as xres_pool,
            tc.tile_pool(name="stg1", bufs=2) as stg1,
            tc.tile_pool(name="small", bufs=2) as small,
            tc.tile_pool(name="psmm", bufs=3, space="PSUM") as psmm,  # 2-bank tiles
            tc.tile_pool(name="psctx", bufs=2, space="PSUM") as psctx,
        ):
            # persistent big tensors; first weight chunk loads interleaved with x^T
            # so the first projection matmuls start as early as possible
            xT_lo = xtp.tile([P, 4, S], fp8, tag="xt", name="xT_lo")
            xT_hi = xtp.tile([P, 4, S], fp8, tag="xt", name="xT_hi")
            wch0 = wp.tile([P, DB, 512], fp8, tag="wp", name="wch0")
            nc.sync.dma_start(wch0[:, 0:4, :], w_d["wq0"][:, 0:4, :])
            for i in range(4):
                nc.sync.dma_start(xT_lo[:, i, :], xt_lo_d[:, i, :])
            nc.sync.dma_start(wch0[:, 4:8, :], w_d["wq0"][:, 4:8, :])
            for i in range(4):
                nc.sync.dma_start(xT_hi[:, i, :], xt_hi_d[:, i, :])

            def xT_pair(j, sl):
                """[P, 2, len] fp8 view of x^T k-tile pair j (db = 2j, 2j+1)."""
                t = xT_lo if j < 2 else xT_hi
                jj = j % 2
                return t[:, 2 * jj : 2 * jj + 2, sl]

            QT = qt_pool.tile([P, DB, S], bf16, tag="qt", name="QT")  # + ctx overlay
            KT = kt_pool.tile([P, DB, S], bf16, tag="kt", name="KT")
            QT8 = qt8_pool.tile([P, DB, S], fp8, tag="qt8", name="QT8")  # 8*ctx fp8
            VP_W = H * (DK + 1) + DK  # 64-col pad so head 15 has a 128-wide window
            Vp = vp_pool.tile([P, SB, VP_W], fp8, tag="vp", name="Vp")

            # ---- constants / singles ----
            bq_sb = singles.tile([P, DB], f32)
            nc.sync.dma_start(bq_sb[:], bqs_d[:, :])
            bk_sb = singles.tile([P, DB], f32)
            nc.sync.dma_start(bk_sb[:], bks_d[:, :])
            eps_sb = singles.tile([P, 1], f32)
            nc.vector.memset(eps_sb[:], EPS)
            ones_f32 = singles.tile([P, 1], f32)
            nc.vector.memset(ones_f32[:], 1.0)

            # epilogue table: cols 0:S s0 (later recip), S:2S bad (later bad*recip).
            # 32 partitions (rows 16-31 stay zero) so it is a clean K=32 rhs for the
            # selector broadcast matmuls.
            tab = singles.tile([2 * H, 2 * S], bf16)
            sumv_all = singles.tile([P, DB], f32)

            # pair-selector for PE-based partition-broadcast of tab rows:
            # pairsel[k, hb*128 + p] = 1 iff k == 2*hb + (p >= 64), host-provided
            pairsel = singles.tile([2 * H, SB * P], bf16)
            nc.sync.dma_start(pairsel[:], pairsel_d[:, :])
            # now zero the whole tab (rows 16-31 must stay zero; rows 0-15 get s0)
            nc.vector.tensor_scalar(
                tab[:, :],
                ones_f32[0 : 2 * H, 0:1].to_broadcast([2 * H, 2 * S]),
                0.0,
                None,
                op0=Alu.mult,
            )

            # ============ Phase 1: projections (fp8 DoubleRow, K=256/pass) ========
            # Q^T / K^T: psum[o_block 128, s 1024] = sum_j WT[pair j, ob].T @ xT[pair j]
            for wname, dst, bias_sb in (("wq", QT, bq_sb), ("wk", KT, bk_sb)):
                for oc in range(2):
                    if wname == "wq" and oc == 0:
                        wch = wch0
                    else:
                        wch = wp.tile([P, DB, 512], fp8, tag="wp", name="wch")
                        nc.sync.dma_start(wch[:, 0:4, :], w_d[f"{wname}{oc}"][:, 0:4, :])
                        nc.sync.dma_start(wch[:, 4:8, :], w_d[f"{wname}{oc}"][:, 4:8, :])
                    for obl in range(4):
                        ob = oc * 4 + obl
                        ps = psmm.tile([P, 2, 512], f32, tag="mm", name="ps_qk")
                        for sc in range(2):
                            for j in range(4):
                                nc.tensor.matmul(
                                    ps[:, sc, :],
                                    lhsT=wch[:, 2 * j : 2 * j + 2, obl * P : (obl + 1) * P],
                                    rhs=xT_pair(j, slice(sc * 512, (sc + 1) * 512)),
                                    start=(j == 0),
                                    stop=(j == 3),
                                    perf_mode=DR,
                                )
                        # rescale + per-partition bias (o on partitions), both chunks
                        nc.scalar.activation(
                            dst[:, ob, :],
                            ps[:].rearrange("p a b -> p (a b)"),
                            Act.Identity,
                            bias=bias_sb[:, ob : ob + 1],
                            scale=INV_XW,
                        )

            # late singles (not needed until V-proj / attention / epilogue)
            padm_sb = singles.tile([P, SB], f32)
            nc.sync.dma_start(padm_sb[:], padm_d[:, :])
            causal_sb = singles.tile([P, P], fp8)
            nc.sync.dma_start(causal_sb[:], causal_d[:, :])
            causx_sb = singles.tile([P, 2 * P], fp8)
            nc.sync.dma_start(causx_sb[:], causx_d[:, :])
            nc.sync.dma_start(sumv_all[:], sumv_d[:, :])
            # padm scaled by the fp8 descale factor, for the V drain
            padm_ds = singles.tile([P, SB], f32)
            nc.vector.tensor_scalar(padm_ds[:], padm_sb[:], INV_XW, None, op0=Alu.mult)
            # V natural: psum[s_block 128, o 512] = sum_j xT[pair j, sb].T @ WvT[pair j]
            for oc in range(2):
                wch = wp.tile([P, DB, 512], fp8, tag="wp", name="wchv")
                nc.sync.dma_start(wch[:, 0:4, :], w_d[f"wv{oc}"][:, 0:4, :])
                nc.sync.dma_start(wch[:, 4:8, :], w_d[f"wv{oc}"][:, 4:8, :])
                for sbi in range(0, SB, 2):
                    ps = psmm.tile([P, 2, 512], f32, tag="mm", name="ps_v")
                    for si in range(2):
                        sb = sbi + si
                        for j in range(4):
                            nc.tensor.matmul(
                                ps[:, si, :],
                                lhsT=xT_pair(j, slice(sb * P, (sb + 1) * P)),
                                rhs=wch[:, 2 * j : 2 * j + 2, :],
                                start=(j == 0),
                                stop=(j == 3),
                                perf_mode=DR,
                            )
                    for si in range(2):
                        sb = sbi + si
                        # scatter into per-head 65-wide slots; rescale + padmask
                        # (on ACT: out = in * scale(per-partition AP))
                        vview = Vp[:, sb, 0 : H * (DK + 1)].rearrange(
                            "p (h c) -> p h c", c=DK + 1
                        )
                        nc.scalar.activation(
                            vview[:, oc * 8 : (oc + 1) * 8, 0:DK],
                            ps[:, si, :].rearrange("p (h c) -> p h c", c=DK),
                            Act.Identity,
                            scale=padm_ds[:, sb : sb + 1],
                        )
            # "ones" columns of V' = padmask (zero for padded keys) + zeroed pad tail
            vv = Vp[:, :, 0 : H * (DK + 1)].rearrange("p sb (h c) -> p sb h c", c=DK + 1)
            nc.vector.tensor_copy(
                vv[:, :, :, DK : DK + 1],
                padm_sb.unsqueeze(2).unsqueeze(3).to_broadcast([P, SB, H, 1]),
            )
            nc.vector.tensor_scalar(
                Vp[:, :, H * (DK + 1) : VP_W],
                ones_f32.unsqueeze(1).to_broadcast([P, SB, DK]),
                0.0,
                None,
                op0=Alu.mult,
            )

            # ============ Phase 2: attention, qc-outer so the qc=0 epilogue +
            # first-half out-projection/LN overlap with qc=1 attention ========
            woch = []

            def emit_norm_out(qc):
                # ---- per-qc softmax normalization epilogue ----
                qch = slice(qc * 512, (qc + 1) * 512)
                T0 = tab[0:H, qc * 512 : (qc + 1) * 512]  # s0 -> denom -> recip
                T2 = tab[0:H, S + qc * 512 : S + (qc + 1) * 512]  # bad*1024
                nc.vector.tensor_scalar(
                    T2, T0, 1e-30, 1024.0, op0=Alu.is_le, op1=Alu.mult
                )
                nc.vector.tensor_tensor(T0, T0, T2, Alu.add)
                with nc.allow_low_precision(
                    reason="recip stored bf16; rounding far below output tolerance"
                ):
                    nc.vector.reciprocal(T0, T0)
                # fold the fp8 ctx scale (x8, dodges e4m3 denormals) into the recip
                nc.vector.tensor_scalar(T0, T0, 8.0, None, op0=Alu.mult)

                for hb in range(8):
                    bc2 = psmm.tile([P, 2, 512], f32, tag="mm", name="bc2")
                    nc.tensor.matmul(
                        bc2[:, 0, :],
                        lhsT=pairsel[:, hb * P : (hb + 1) * P],
                        rhs=tab[:, S + qc * 512 : S + (qc + 1) * 512],
                        start=True,
                        stop=True,
                    )
                    nc.tensor.matmul(
                        bc2[:, 1, :],
                        lhsT=pairsel[:, hb * P : (hb + 1) * P],
                        rhs=tab[:, qc * 512 : (qc + 1) * 512],
                        start=True,
                        stop=True,
                    )
                    bcp = bc2[:, 0, :]
                    rc = bc2[:, 1, :]
                    # ctx = (ctx_u + bad1024 * sumV/1024) * recip (whole pair)
                    nc.vector.scalar_tensor_tensor(
                        QT[:, hb, qch],
                        bcp,
                        sumv_all[:, hb : hb + 1],
                        QT[:, hb, qch],
                        op0=Alu.mult,
                        op1=Alu.add,
                    )
                    nc.vector.tensor_tensor(
                        QT8[:, hb, qch], QT[:, hb, qch], rc, Alu.mult
                    )

                # ---- out-projection + residual + LayerNorm for this q-half ----
                for sb in range(qc * 4, qc * 4 + 4):
                    xres = xres_pool.tile([P, D], f32, tag="xres", name="xres")
                    nc.sync.dma_start(xres[:], xr_d[sb * P : (sb + 1) * P, :])
                    res = xres  # residual-add and LN happen in place
                    ps = psmm.tile([P, 2, 512], f32, tag="mm", name="ps_o")
                    for oc in range(2):
                        for j in range(4):
                            nc.tensor.matmul(
                                ps[:, oc, :],
                                lhsT=QT8[:, 2 * j : 2 * j + 2, sb * P : (sb + 1) * P],
                                rhs=woch[oc][:, 2 * j : 2 * j + 2, :],
                                start=(j == 0),
                                stop=(j == 3),
                                perf_mode=DR,
                            )
                    # descale (ctx x8, Wo x256) + residual in one pass
                    nc.vector.scalar_tensor_tensor(
                        res[:, :],
                        ps[:].rearrange("p a b -> p (a b)"),
                        1.0 / (8.0 * WS),
                        xres[:, :],
                        op0=Alu.mult,
                        op1=Alu.add,
                    )
                    # LayerNorm over free dim (1024) via bn_stats (2 subgroups)
                    stats = small.tile([P, 2, 6], f32, tag="stats", name="stats")
                    nc.vector.bn_stats(stats[:, 0, :], res[:, 0:512])
                    nc.vector.bn_stats(stats[:, 1, :], res[:, 512:1024])
                    mv = small.tile([P, 2], f32, tag="mv", name="mv")
                    nc.vector.bn_aggr(mv[:], stats[:])
                    rstd = small.tile([P, 1], f32, tag="rstd", name="rstd")
                    nc.scalar.activation(
                        rstd[:], mv[:, 1:2], Act.Sqrt, bias=eps_sb[:], scale=1.0
                    )
                    nc.vector.reciprocal(rstd[:], rstd[:])
                    nc.vector.tensor_scalar(
                        res[:], res[:], mv[:, 0:1], rstd[:],
                        op0=Alu.subtract, op1=Alu.mult,
                    )
                    nc.sync.dma_start(out_d[sb * P : (sb + 1) * P, :], res[:])

            for qc in range(2):
                for hb in range(8):
                    uts = {}
                    for par in range(2):
                        ut = up.tile([P, DB, 512], fp8, tag="up", name=f"ut{par}")
                        uts[par] = ut
                    for kb0, kb1 in _kbs(qc):
                        vs = _vs(kb0, qc)  # pair shares the lower block's start col
                        for par in range(2):
                            hp = 64 * par
                            ps = psmm.tile([P, 2, 512], f32, tag="mm", name="ps_sc")
                            for i, kb in enumerate((kb0, kb1)):
                                nc.tensor.matmul(
                                    ps[:, i, vs:512],
                                    lhsT=KT[hp : hp + DK, hb, kb * P : (kb + 1) * P],
                                    rhs=QT[
                                        hp : hp + DK, hb, qc * 512 + vs : qc * 512 + 512
                                    ],
                                    start=True,
                                    stop=True,
                                )
                            # u = exp(0.125*scores); padding handled by zeroed V rows
                            upair = uts[par][:, kb0 : kb0 + 2, vs:512]
                            nc.scalar.activation(
                                upair, ps[:, :, vs:512], Act.Exp, scale=SCALE
                            )
                            if kb0 * P >= qc * 512:  # diagonal region: mask on u
                                # lower block: 0/1 triangle on its crossing square
                                nc.vector.tensor_mul(
                                    uts[par][:, kb0, vs : vs + P],
                                    uts[par][:, kb0, vs : vs + P],
                                    causal_sb[:],
                                )
                                # upper block: [zeros|triangle] over its garbage
                                # window + crossing square (it shares vs of kb0)
                                nc.vector.tensor_mul(
                                    uts[par][:, kb1, vs : vs + 2 * P],
                                    uts[par][:, kb1, vs : vs + 2 * P],
                                    causx_sb[:],
                                )
                    for par in range(2):
                        h = 2 * hb + par
                        cps = psctx.tile([P, 512], f32, tag="ctx", name="cps")
                        pairs = _kbs(qc)
                        for i, (kb0, kb1) in enumerate(pairs):
                            vs = _vs(kb0, qc)
                            nc.tensor.matmul(
                                cps[:, vs:512],
                                lhsT=Vp[:, kb0 : kb0 + 2, h * (DK + 1) : h * (DK + 1) + P],
                                rhs=uts[par][:, kb0 : kb0 + 2, vs:512],
                                start=(i == 0),
                                stop=(i == len(pairs) - 1),
                                perf_mode=DR,
                            )
                        # drain ctx + s0; s0 always staged via stg (a QT row 64
                        # staging spot would race the odd head's gpsimd write)
                        if par == 0:
                            nc.scalar.activation(
                                QT[0:DK, hb, qc * 512 : (qc + 1) * 512],
                                cps[0:DK, 0:512],
                                Act.Identity,
                            )
                            stge = stg1.tile([P, 512], bf16, tag="stg", name="stge")
                            nc.vector.tensor_copy(
                                stge[DK : DK + 1, 0:512], cps[DK : DK + 1, 0:512]
                            )
                            nc.sync.dma_start(
                                tab[h : h + 1, qc * 512 : (qc + 1) * 512],
                                stge[DK : DK + 1, 0:512],
                            )
                        else:
                            stg = stg1.tile([P, 512], bf16, tag="stg", name="stg")
                            nc.vector.tensor_copy(
                                stg[0 : DK + 1, 0:512], cps[0 : DK + 1, 0:512]
                            )
                            nc.sync.dma_start(
                                tab[h : h + 1, qc * 512 : (qc + 1) * 512],
                                stg[DK : DK + 1, 0:512],
                            )
                            nc.gpsimd.tensor_copy(
                                QT[DK:P, hb, qc * 512 : (qc + 1) * 512],
                                stg[0:DK, 0:512],
                            )

                    if qc == 1 and hb == 1:
                        # qc0's normalization + out-proj/LN, emitted two head
                        # groups into qc1 so its PE work fills queue bubbles
                        emit_norm_out(0)

                if qc == 0:
                    # prefetch Wo chunks (used by the out-proj)
                    for oc in range(2):
                        wch = wop.tile([P, DB, 512], fp8, tag="wop", name="woch")
                        nc.sync.dma_start(wch[:, 0:4, :], w_d[f"wo{oc}"][:, 0:4, :])
                        nc.sync.dma_start(wch[:, 4:8, :], w_d[f"wo{oc}"][:, 4:8, :])
                        woch.append(wch)

            emit_norm_out(1)

    nc.compile()
    return nc


def _stripe_w(WT):
    """[D, D] (d_in, d_out) -> two contiguous [P, DB, 512] o-half chunks."""
    a = np.ascontiguousarray(WT.reshape(DB, P, D).transpose(1, 0, 2))  # [p, db, o]
    return (
        np.ascontiguousarray(a[:, :, 0:512]),
        np.ascontiguousarray(a[:, :, 512:1024]),
    )


def kernel(
    history_items,
    sequence_mask,
    Wq,
    bq,
    Wk,
    bk,
    Wv,
    bv,
    Wo,
    bo,
    ln_gamma,
    ln_beta,
):
    from concourse.bass_utils import run_bass_kernel_spmd

    global _built
    if _built is None:
        _built = _build()
    nc = _built

    import ml_dtypes

    bf16 = ml_dtypes.bfloat16
    fp8 = ml_dtypes.float8_e4m3
    x = np.asarray(history_items, dtype=np.float32)
    mask = np.asarray(sequence_mask)
    f = lambda a: np.ascontiguousarray(np.asarray(a, dtype=np.float32))
    fb = lambda a: np.ascontiguousarray(np.asarray(a, dtype=np.float32).astype(bf16))
    f8 = lambda a: np.ascontiguousarray(np.asarray(a, dtype=np.float32).astype(fp8))

    common = {}
    for wname, W in (("wq", Wq), ("wk", Wk), ("wv", Wv), ("wo", Wo)):
        c0, c1 = _stripe_w(f(np.asarray(W).T * WS))
        common[f"{wname}0"] = f8(c0)
        common[f"{wname}1"] = f8(c1)
    common["bqs"] = f(np.asarray(bq).reshape(DB, P).T)
    common["bks"] = f(np.asarray(bk).reshape(DB, P).T)
    k_idx = np.arange(2 * H)[:, None]
    hb_idx = np.repeat(np.arange(SB), P)[None, :]
    c1_idx = np.tile((np.arange(P) >= 64).astype(np.int64), SB)[None, :]
    common["pairsel"] = fb((k_idx == 2 * hb_idx + c1_idx).astype(np.float32))
    tri = np.where(
        np.arange(P)[None, :] >= np.arange(P)[:, None], 1.0, 0.0
    ).astype(np.float32)
    common["causal"] = f8(tri)
    common["causx"] = f8(np.concatenate([np.zeros((P, P), np.float32), tri], axis=1))
    # attn-output bias bv contributes bv @ Wo.T (constant over s) -> fold into residual
    bo_row = (
        np.asarray(bo, dtype=np.float64)
        + np.asarray(bv, dtype=np.float64) @ np.asarray(Wo, dtype=np.float64).T
    ).astype(np.float32)

    in_maps = []
    for b in range(N_CORES):
        xT = f(x[b].T * XS).astype(fp8).reshape(DB, P, S).transpose(1, 0, 2)
        pm = (mask[b] != 0).astype(np.float32)
        sx = x[b].astype(np.float64).sum(axis=0)
        sumv = ((sx @ np.asarray(Wv, dtype=np.float64).T) / 1024.0).astype(np.float32)
        in_maps.append(
            {
                **common,
                "xt_lo": np.ascontiguousarray(xT[:, 0:4, :]),
                "xt_hi": np.ascontiguousarray(xT[:, 4:8, :]),
                "xr": f(x[b] + bo_row[None, :]),
                "padm": f(pm.reshape(SB, P).T),
                "sumv": f(sumv.reshape(DB, P).T),
            }
        )

    r = run_bass_kernel_spmd(nc, in_maps, core_ids=list(range(N_CORES)))
    out = np.stack([res["out"] for res in r.results]).astype(np.float32)

    g = np.asarray(ln_gamma, dtype=np.float32)
    be = np.asarray(ln_beta, dtype=np.float32)
    out = out * g[None, None, :] + be[None, None, :]
    return out.astype(np.float32)


# revision 25
# speedup vs baseline: 1.1410x; 1.1410x over previous
"""Trainium2 Bass kernel for causal multi-head attention block (B=8, S=1024, D=1024, H=16).

Sharding: pure batch data-parallelism - one batch element per NeuronCore (B=8, 8 cores).
Each core runs the full transformer block on its [S, D] slice; no collectives.

Per-core algorithm (layouts chosen so no on-device transposes are needed):
  - Host passes x^T and all W^T pre-strided into the SBUF partition layout
    [p, db, cols] so every big DMA is 128 large contiguous descriptors.
  - QKV projections run in fp8e4 DoubleRow mode (two 128-deep k-tiles per pass,
    so K=256 per matmul at bf16-rate): host scales x by 8 and Wq/Wk/Wv by 256
    (keeps U(-1/32,1/32) weights out of fp8 denormals); the PSUM drain rescales
    by 1/2048 and adds the bias.
  - Q^T, K^T land as [o, s] bf16; per-head [dk, S] slices feed the scores matmul.
  - V lands natural [s, o] in fp8 per head with an appended ones column
    (V' = [V | padmask]) so the ctx matmul also yields the softmax denominator
    s0 in PSUM row 64.  Padded keys have V rows zeroed.
  - scoresT[k, q] = matmul(lhsT=K^T head slice, rhs=Q^T head slice) in bf16
    (contraction dk=64; even/odd heads at PE row groups 0/64 run concurrently).
    k-blocks processed in pairs sharing one 2-bank PSUM tile.
  - Softmax without max-subtraction: u = exp(0.125*scores) on ACT, written as
    fp8.  Causal masking multiplies u by 0/1 masks on DVE: a [128,128] triangle
    for the pair's lower block, and a [128,256] [zeros|triangle] extended mask
    for the upper block (which shares the lower block's column window, so its
    below-diagonal garbage must be zeroed for the DoubleRow ctx matmul).
  - ctx matmul in fp8 DoubleRow over k-block pairs (K=256 per pass):
    lhsT=V' pair, rhs=u pair.
  - Rows with a fully-masked causal window (s0 == 0) must match the reference's
    softmax(-1e9*ones) = uniform over ALL 1024 keys: ctx = (ctx_u + bad*sumV)/
    (s0 + 1024*bad), bad = (s0 <= 1e-30), sumV = column sums of V'.
  - The per-(head,q) normalizers are broadcast across partitions with a selector
    matmul (lhsT = 0/1 pair-selector, rhs = the 32-row table) into PSUM.
  - Out projection in bf16: out[s, o] = matmul(lhsT=ctx^T block, rhs=Wo^T);
    residual x+bo added, LayerNorm via bn_stats/bn_aggr.  gamma/beta applied on
    host (exact no-op for the reference's ones/zeros).
"""

import sys

import numpy as np

if "/opt/trn_rl_repo" not in sys.path:
    sys.path.insert(0, "/opt/trn_rl_repo")

S = 1024
D = 1024
H = 16
DK = 64
P = 128
DB = D // P  # 8 d-blocks
SB = S // P  # 8 s-blocks
NEG = -1.0e9
SCALE = 0.125  # 1/sqrt(64)
EPS = 1e-5
N_CORES = 8
XS = 8.0  # host scale on x for fp8
WS = 256.0  # host scale on Wq/Wk/Wv for fp8
INV_XW = 1.0 / (XS * WS)

_built = None


def _kbs(qc):
    """k-block pairs needed for q-chunk qc (q in [qc*512, qc*512+512))."""
    return [(0, 1), (2, 3)] if qc == 0 else [(0, 1), (2, 3), (4, 5), (6, 7)]


def _vs(kb, qc):
    """first causal q column within the 512-wide chunk for k-block kb."""
    return max(0, kb * P - qc * 512)


def _build():
    import concourse.mybir as mybir
    import concourse.tile as tile
    from concourse import bacc

    f32 = mybir.dt.float32
    bf16 = mybir.dt.bfloat16
    fp8 = mybir.dt.float8e4
    Alu = mybir.AluOpType
    Act = mybir.ActivationFunctionType
    DR = mybir.MatmulPerfMode.DoubleRow

    nc = bacc.Bacc()

    # ---- DRAM I/O (pre-strided [p, db, cols] contiguous layouts from host) ----
    xt_lo_d = nc.dram_tensor("xt_lo", [P, 4, S], fp8, kind="ExternalInput")
    xt_hi_d = nc.dram_tensor("xt_hi", [P, 4, S], fp8, kind="ExternalInput")
    xr_d = nc.dram_tensor("xr", [S, D], f32, kind="ExternalInput")  # x + bo
    w_d = {}
    for wname in ("wq", "wk", "wv", "wo"):
        for oc in range(2):
            n = f"{wname}{oc}"
            w_d[n] = nc.dram_tensor(n, [P, DB, 512], fp8, kind="ExternalInput")
    bqs_d = nc.dram_tensor("bqs", [P, DB], f32, kind="ExternalInput")
    bks_d = nc.dram_tensor("bks", [P, DB], f32, kind="ExternalInput")
    padm_d = nc.dram_tensor("padm", [P, SB], f32, kind="ExternalInput")  # 1 valid / 0 pad
    sumv_d = nc.dram_tensor("sumv", [P, DB], f32, kind="ExternalInput")
    pairsel_d = nc.dram_tensor("pairsel", [2 * H, SB * P], bf16, kind="ExternalInput")
    causal_d = nc.dram_tensor("causal", [P, P], fp8, kind="ExternalInput")  # 0/1
    causx_d = nc.dram_tensor("causx", [P, 2 * P], fp8, kind="ExternalInput")  # [0|tri]
    out_d = nc.dram_tensor("out", [S, D], f32, kind="ExternalOutput")

    with tile.TileContext(nc) as tc:
        with (
            tc.tile_pool(name="singles", bufs=1) as singles,
            tc.tile_pool(name="qt", bufs=1) as qt_pool,
            tc.tile_pool(name="kt", bufs=1) as kt_pool,
            tc.tile_pool(name="vp", bufs=1) as vp_pool,
            tc.tile_pool(name="xtp", bufs=2) as xtp,  # two halves of x^T (fp8)
            tc.tile_pool(name="wp", bufs=3) as wp,  # fp8 qkv weight chunks
            tc.tile_pool(name="up", bufs=4) as up,  # fp8 u chunks (2 live/iter)
            tc.tile_pool(name="wop", bufs=2) as wop,  # fp8 wo chunks
            tc.tile_pool(name="qt8", bufs=1) as qt8_pool,  # fp8 normalized ctx
            tc.tile_pool(name="xres", bufs=2) as xres_pool,
            tc.tile_pool(name="stg1", bufs=2) as stg1,
            tc.tile_pool(name="small", bufs=2) as small,
            tc.tile_pool(name="psmm", bufs=3, space="PSUM") as psmm,  # 2-bank tiles
            tc.tile_pool(name="psctx", bufs=2, space="PSUM") as psctx,
        ):
            # persistent big tensors; first weight chunk loads interleaved with x^T
            # so the first projection matmuls start as early as possible
            xT_lo = xtp.tile([P, 4, S], fp8, tag="xt", name="xT_lo")
            xT_hi = xtp.tile([P, 4, S], fp8, tag="xt", name="xT_hi")
            wch0 = wp.tile([P, DB, 512], fp8, tag="wp", name="wch0")
            nc.sync.dma_start(wch0[:, 0:4, :], w_d["wq0"][:, 0:4, :])
            for i in range(4):
                nc.sync.dma_start(xT_lo[:, i, :], xt_lo_d[:, i, :])
            nc.sync.dma_start(wch0[:, 4:8, :], w_d["wq0"][:, 4:8, :])
            for i in range(4):
                nc.sync.dma_start(xT_hi[:, i, :], xt_hi_d[:, i, :])

            def xT_pair(j, sl):
                """[P, 2, len] fp8 view of x^T k-tile pair j (db = 2j, 2j+1)."""
                t = xT_lo if j < 2 else xT_hi
                jj = j % 2
                return t[:, 2 * jj : 2 * jj + 2, sl]

            QT = qt_pool.tile([P, DB, S], bf16, tag="qt", name="QT")  # + ctx overlay
            KT = kt_pool.tile([P, DB, S], bf16, tag="kt", name="KT")
            QT8 = qt8_pool.tile([P, DB, S], fp8, tag="qt8", name="QT8")  # 8*ctx fp8
            VP_W = H * (DK + 1) + DK  # 64-col pad so head 15 has a 128-wide window
            Vp = vp_pool.tile([P, SB, VP_W], fp8, tag="vp", name="Vp")

            # ---- constants / singles ----
            bq_sb = singles.tile([P, DB], f32)
            nc.sync.dma_start(bq_sb[:], bqs_d[:, :])
            bk_sb = singles.tile([P, DB], f32)
            nc.sync.dma_start(bk_sb[:], bks_d[:, :])
            eps_sb = singles.tile([P, 1], f32)
            nc.vector.memset(eps_sb[:], EPS)
            ones_f32 = singles.tile([P, 1], f32)
            nc.vector.memset(ones_f32[:], 1.0)

            # epilogue table: cols 0:S s0 (later recip), S:2S bad (later bad*recip).
            # 32 partitions (rows 16-31 stay zero) so it is a clean K=32 rhs for the
            # selector broadcast matmuls.
            tab = singles.tile([2 * H, 2 * S], bf16)
            sumv_all = singles.tile([P, DB], f32)

            # pair-selector for PE-based partition-broadcast of tab rows:
            # pairsel[k, hb*128 + p] = 1 iff k == 2*hb + (p >= 64), host-provided
            pairsel = singles.tile([2 * H, SB * P], bf16)
            nc.sync.dma_start(pairsel[:], pairsel_d[:, :])
            # now zero the whole tab (rows 16-31 must stay zero; rows 0-15 get s0)
            nc.vector.tensor_scalar(
                tab[:, :],
                ones_f32[0 : 2 * H, 0:1].to_broadcast([2 * H, 2 * S]),
                0.0,
                None,
                op0=Alu.mult,
            )

            # ============ Phase 1: projections (fp8 DoubleRow, K=256/pass) ========
            # Q^T / K^T: psum[o_block 128, s 1024] = sum_j WT[pair j, ob].T @ xT[pair j]
            for wname, dst, bias_sb in (("wq", QT, bq_sb), ("wk", KT, bk_sb)):
                for oc in range(2):
                    if wname == "wq" and oc == 0:
                        wch = wch0
                    else:
                        wch = wp.tile([P, DB, 512], fp8, tag="wp", name="wch")
                        nc.sync.dma_start(wch[:, 0:4, :], w_d[f"{wname}{oc}"][:, 0:4, :])
                        nc.sync.dma_start(wch[:, 4:8, :], w_d[f"{wname}{oc}"][:, 4:8, :])
                    for obl in range(4):
                        ob = oc * 4 + obl
                        ps = psmm.tile([P, 2, 512], f32, tag="mm", name="ps_qk")
                        for sc in range(2):
                            for j in range(4):
                                nc.tensor.matmul(
                                    ps[:, sc, :],
                                    lhsT=wch[:, 2 * j : 2 * j + 2, obl * P : (obl + 1) * P],
                                    rhs=xT_pair(j, slice(sc * 512, (sc + 1) * 512)),
                                    start=(j == 0),
                                    stop=(j == 3),
                                    perf_mode=DR,
                                )
                        # rescale + per-partition bias (o on partitions), both chunks
                        nc.scalar.activation(
                            dst[:, ob, :],
                            ps[:].rearrange("p a b -> p (a b)"),
                            Act.Identity,
                            bias=bias_sb[:, ob : ob + 1],
                            scale=INV_XW,
                        )

            # late singles (not needed until V-proj / attention / epilogue)
            padm_sb = singles.tile([P, SB], f32)
            nc.sync.dma_start(padm_sb[:], padm_d[:, :])
            causal_sb = singles.tile([P, P], fp8)
            nc.sync.dma_start(causal_sb[:], causal_d[:, :])
            causx_sb = singles.tile([P, 2 * P], fp8)
            nc.sync.dma_start(causx_sb[:], causx_d[:, :])
            nc.sync.dma_start(sumv_all[:], sumv_d[:, :])
            # padm scaled by the fp8 descale factor, for the V drain
            padm_ds = singles.tile([P, SB], f32)
            nc.vector.tensor_scalar(padm_ds[:], padm_sb[:], INV_XW, None, op0=Alu.mult)
            # V natural: psum[s_block 128, o 512] = sum_j xT[pair j, sb].T @ WvT[pair j]
            for oc in range(2):
                wch = wp.tile([P, DB, 512], fp8, tag="wp", name="wchv")
                nc.sync.dma_start(wch[:, 0:4, :], w_d[f"wv{oc}"][:, 0:4, :])
                nc.sync.dma_start(wch[:, 4:8, :], w_d[f"wv{oc}"][:, 4:8, :])
                for sbi in range(0, SB, 2):
                    ps = psmm.tile([P, 2, 512], f32, tag="mm", name="ps_v")
                    for si in range(2):
                        sb = sbi + si
                        for j in range(4):
                            nc.tensor.matmul(
                                ps[:, si, :],
                                lhsT=xT_pair(j, slice(sb * P, (sb + 1) * P)),
                                rhs=wch[:, 2 * j : 2 * j + 2, :],
                                start=(j == 0),
                                stop=(j == 3),
                                perf_mode=DR,
                            )
                    for si in range(2):
                        sb = sbi + si
                        # scatter into per-head 65-wide slots; rescale + padmask
                        # (on ACT: out = in * scale(per-partition AP))
                        vview = Vp[:, sb, 0 : H * (DK + 1)].rearrange(
                            "p (h c) -> p h c", c=DK + 1
                        )
                        nc.scalar.activation(
                            vview[:, oc * 8 : (oc + 1) * 8, 0:DK],
                            ps[:, si, :].rearrange("p (h c) -> p h c", c=DK),
                            Act.Identity,
                            scale=padm_ds[:, sb : sb + 1],
                        )
            # "ones" columns of V' = padmask (zero for padded keys) + zeroed pad tail
            vv = Vp[:, :, 0 : H * (DK + 1)].rearrange("p sb (h c) -> p sb h c", c=DK + 1)
            nc.vector.tensor_copy(
                vv[:, :, :, DK : DK + 1],
                padm_sb.unsqueeze(2).unsqueeze(3).to_broadcast([P, SB, H, 1]),
            )
            nc.vector.tensor_scalar(
                Vp[:, :, H * (DK + 1) : VP_W],
                ones_f32.unsqueeze(1).to_broadcast([P, SB, DK]),
                0.0,
                None,
                op0=Alu.mult,
            )

            # ============ Phase 2: attention, qc-outer so the qc=0 epilogue +
            # first-half out-projection/LN overlap with qc=1 attention ========
            woch = []

            def emit_norm_out(qc):
                # ---- per-qc softmax normalization epilogue ----
                qch = slice(qc * 512, (qc + 1) * 512)
                T0 = tab[0:H, qc * 512 : (qc + 1) * 512]  # s0 -> denom -> recip
                T2 = tab[0:H, S + qc * 512 : S + (qc + 1) * 512]  # bad*1024
                nc.vector.tensor_scalar(
                    T2, T0, 1e-30, 1024.0, op0=Alu.is_le, op1=Alu.mult
                )
                nc.vector.tensor_tensor(T0, T0, T2, Alu.add)
                with nc.allow_low_precision(
                    reason="recip stored bf16; rounding far below output tolerance"
                ):
                    nc.vector.reciprocal(T0, T0)
                # fold the fp8 ctx scale (x8, dodges e4m3 denormals) into the recip
                nc.vector.tensor_scalar(T0, T0, 8.0, None, op0=Alu.mult)

                for hb in range(8):
                    bc2 = psmm.tile([P, 2, 512], f32, tag="mm", name="bc2")
                    nc.tensor.matmul(
                        bc2[:, 0, :],
                        lhsT=pairsel[:, hb * P : (hb + 1) * P],
                        rhs=tab[:, S + qc * 512 : S + (qc + 1) * 512],
                        start=True,
                        stop=True,
                    )
                    nc.tensor.matmul(
                        bc2[:, 1, :],
                        lhsT=pairsel[:, hb * P : (hb + 1) * P],
                        rhs=tab[:, qc * 512 : (qc + 1) * 512],
                        start=True,
                        stop=True,
                    )
                    bcp = bc2[:, 0, :]
                    rc = bc2[:, 1, :]
                    # ctx = (ctx_u + bad1024 * sumV/1024) * recip (whole pair)
                    nc.vector.scalar_tensor_tensor(
                        QT[:, hb, qch],
                        bcp,
                        sumv_all[:, hb : hb + 1],
                        QT[:, hb, qch],
                        op0=Alu.mult,
                        op1=Alu.add,
                    )
                    nc.vector.tensor_tensor(
                        QT8[:, hb, qch], QT[:, hb, qch], rc, Alu.mult
                    )

                # ---- out-projection + residual + LayerNorm for this q-half ----
                for sb in range(qc * 4, qc * 4 + 4):
                    xres = xres_pool.tile([P, D], f32, tag="xres", name="xres")
                    nc.sync.dma_start(xres[:], xr_d[sb * P : (sb + 1) * P, :])
                    res = xres  # residual-add and LN happen in place
                    ps = psmm.tile([P, 2, 512], f32, tag="mm", name="ps_o")
                    for oc in range(2):
                        for j in range(4):
                            nc.tensor.matmul(
                                ps[:, oc, :],
                                lhsT=QT8[:, 2 * j : 2 * j + 2, sb * P : (sb + 1) * P],
                                rhs=woch[oc][:, 2 * j : 2 * j + 2, :],
                                start=(j == 0),
                                stop=(j == 3),
                                perf_mode=DR,
                            )
                    # descale (ctx x8, Wo x256) + residual in one pass
                    nc.vector.scalar_tensor_tensor(
                        res[:, :],
                        ps[:].rearrange("p a b -> p (a b)"),
                        1.0 / (8.0 * WS),
                        xres[:, :],
                        op0=Alu.mult,
                        op1=Alu.add,
                    )
                    # LayerNorm over free dim (1024) via bn_stats (2 subgroups)
                    stats = small.tile([P, 2, 6], f32, tag="stats", name="stats")
                    nc.vector.bn_stats(stats[:, 0, :], res[:, 0:512])
                    nc.vector.bn_stats(stats[:, 1, :], res[:, 512:1024])
                    mv = small.tile([P, 2], f32, tag="mv", name="mv")
                    nc.vector.bn_aggr(mv[:], stats[:])
                    rstd = small.tile([P, 1], f32, tag="rstd", name="rstd")
                    nc.scalar.activation(
                        rstd[:], mv[:, 1:2], Act.Sqrt, bias=eps_sb[:], scale=1.0
                    )
                    nc.vector.reciprocal(rstd[:], rstd[:])
                    nc.vector.tensor_scalar(
                        res[:], res[:], mv[:, 0:1], rstd[:],
                        op0=Alu.subtract, op1=Alu.mult,
                    )
                    nc.sync.dma_start(out_d[sb * P : (sb + 1) * P, :], res[:])

            for qc in range(2):
                for hb in range(8):
                    uts = {}
                    for par in range(2):
                        ut = up.tile([P, DB, 512], fp8, tag="up", name=f"ut{par}")
                        uts[par] = ut
                    for kb0, kb1 in _kbs(qc):
                        vs = _vs(kb0, qc)  # pair shares the lower block's start col
                        for par in range(2):
                            hp = 64 * par
                            ps = psmm.tile([P, 2, 512], f32, tag="mm", name="ps_sc")
                            for i, kb in enumerate((kb0, kb1)):
                                nc.tensor.matmul(
                                    ps[:, i, vs:512],
                                    lhsT=KT[hp : hp + DK, hb, kb * P : (kb + 1) * P],
                                    rhs=QT[
                                        hp : hp + DK, hb, qc * 512 + vs : qc * 512 + 512
                                    ],
                                    start=True,
                                    stop=True,
                                )
                            # u = exp(0.125*scores); padding handled by zeroed V rows
                            upair = uts[par][:, kb0 : kb0 + 2, vs:512]
                            nc.scalar.activation(
                                upair, ps[:, :, vs:512], Act.Exp, scale=SCALE
                            )
                            if kb0 * P >= qc * 512:  # diagonal region: mask on u
                                # lower block: 0/1 triangle on its crossing square
                                nc.vector.tensor_mul(
                                    uts[par][:, kb0, vs : vs + P],
                                    uts[par][:, kb0, vs : vs + P],
                                    causal_sb[:],
                                )
                                # upper block: [zeros|triangle] over its garbage
                                # window + crossing square (it shares vs of kb0)
                                nc.vector.tensor_mul(
                                    uts[par][:, kb1, vs : vs + 2 * P],
                                    uts[par][:, kb1, vs : vs + 2 * P],
                                    causx_sb[:],
                                )
                    for par in range(2):
                        h = 2 * hb + par
                        cps = psctx.tile([P, 512], f32, tag="ctx", name="cps")
                        pairs = _kbs(qc)
                        for i, (kb0, kb1) in enumerate(pairs):
                            vs = _vs(kb0, qc)
                            nc.tensor.matmul(
                                cps[:, vs:512],
                                lhsT=Vp[:, kb0 : kb0 + 2, h * (DK + 1) : h * (DK + 1) + P],
                                rhs=uts[par][:, kb0 : kb0 + 2, vs:512],
                                start=(i == 0),
                                stop=(i == len(pairs) - 1),
                                perf_mode=DR,
                            )
                        # drain ctx + s0; s0 always staged via stg (a QT row 64
                        # staging spot would race the odd head's gpsimd write)
                        if par == 0:
                            nc.scalar.activation(
                                QT[0:DK, hb, qc * 512 : (qc + 1) * 512],
                                cps[0:DK, 0:512],
                                Act.Identity,
                            )
                            stge = stg1.tile([P, 512], bf16, tag="stg", name="stge")
                            nc.vector.tensor_copy(
                                stge[DK : DK + 1, 0:512], cps[DK : DK + 1, 0:512]
                            )
                            nc.sync.dma_start(
                                tab[h : h + 1, qc * 512 : (qc + 1) * 512],
                                stge[DK : DK + 1, 0:512],
                            )
                        else:
                            stg = stg1.tile([P, 512], bf16, tag="stg", name="stg")
                            nc.vector.tensor_copy(
                                stg[0 : DK + 1, 0:512], cps[0 : DK + 1, 0:512]
                            )
                            nc.sync.dma_start(
                                tab[h : h + 1, qc * 512 : (qc + 1) * 512],
                                stg[DK : DK + 1, 0:512],
                            )
                            nc.gpsimd.tensor_copy(
                                QT[DK:P, hb, qc * 512 : (qc + 1) * 512],
                                stg[0:DK, 0:512],
                            )

                if qc == 0:
                    # prefetch Wo chunks (used by the out-proj)
                    for oc in range(2):
                        wch = wop.tile([P, DB, 512], fp8, tag="wop", name="woch")
                        nc.sync.dma_start(wch[:, 0:4, :], w_d[f"wo{oc}"][:, 0:4, :])
                        nc.sync.dma_start(wch[:, 4:8, :], w_d[f"wo{oc}"][:, 4:8, :])
                        woch.append(wch)
                emit_norm_out(qc)

    nc.compile()
    return nc


def _stripe_w(WT):
    """[D, D] (d_in, d_out) -> two contiguous [P, DB, 512] o-half chunks."""
    a = np.ascontiguousarray(WT.reshape(DB, P, D).transpose(1, 0, 2))  # [p, db, o]
    return (
        np.ascontiguousarray(a[:, :, 0:512]),
        np.ascontiguousarray(a[:, :, 512:1024]),
    )


def kernel(
    history_items,
    sequence_mask,
    Wq,
    bq,
    Wk,
    bk,
    Wv,
    bv,
    Wo,
    bo,
    ln_gamma,
    ln_beta,
):
    from concourse.bass_utils import run_bass_kernel_spmd

    global _built
    if _built is None:
        _built = _build()
    nc = _built

    import ml_dtypes

    bf16 = ml_dtypes.bfloat16
    fp8 = ml_dtypes.float8_e4m3
    x = np.asarray(history_items, dtype=np.float32)
    mask = np.asarray(sequence_mask)
    f = lambda a: np.ascontiguousarray(np.asarray(a, dtype=np.float32))
    fb = lambda a: np.ascontiguousarray(np.asarray(a, dtype=np.float32).astype(bf16))
    f8 = lambda a: np.ascontiguousarray(np.asarray(a, dtype=np.float32).astype(fp8))

    common = {}
    for wname, W in (("wq", Wq), ("wk", Wk), ("wv", Wv), ("wo", Wo)):
        c0, c1 = _stripe_w(f(np.asarray(W).T * WS))
        common[f"{wname}0"] = f8(c0)
        common[f"{wname}1"] = f8(c1)
    common["bqs"] = f(np.asarray(bq).reshape(DB, P).T)
    common["bks"] = f(np.asarray(bk).reshape(DB, P).T)
    k_idx = np.arange(2 * H)[:, None]
    hb_idx = np.repeat(np.arange(SB), P)[None, :]
    c1_idx = np.tile((np.arange(P) >= 64).astype(np.int64), SB)[None, :]
    common["pairsel"] = fb((k_idx == 2 * hb_idx + c1_idx).astype(np.float32))
    tri = np.where(
        np.arange(P)[None, :] >= np.arange(P)[:, None], 1.0, 0.0
    ).astype(np.float32)
    common["causal"] = f8(tri)
    common["causx"] = f8(np.concatenate([np.zeros((P, P), np.float32), tri], axis=1))
    # attn-output bias bv contributes bv @ Wo.T (constant over s) -> fold into residual
    bo_row = (
        np.asarray(bo, dtype=np.float64)
        + np.asarray(bv, dtype=np.float64) @ np.asarray(Wo, dtype=np.float64).T
    ).astype(np.float32)

    in_maps = []
    for b in range(N_CORES):
        xT = f(x[b].T * XS).astype(fp8).reshape(DB, P, S).transpose(1, 0, 2)
        pm = (mask[b] != 0).astype(np.float32)
        sx = x[b].astype(np.float64).sum(axis=0)
        sumv = ((sx @ np.asarray(Wv, dtype=np.float64).T) / 1024.0).astype(np.float32)
        in_maps.append(
            {
                **common,
                "xt_lo": np.ascontiguousarray(xT[:, 0:4, :]),
                "xt_hi": np.ascontiguousarray(xT[:, 4:8, :]),
                "xr": f(x[b] + bo_row[None, :]),
                "padm": f(pm.reshape(SB, P).T),
                "sumv": f(sumv.reshape(DB, P).T),
            }
        )

    r = run_bass_kernel_spmd(nc, in_maps, core_ids=list(range(N_CORES)))
    out = np.stack([res["out"] for res in r.results]).astype(np.float32)

    g = np.asarray(ln_gamma, dtype=np.float32)
    be = np.asarray(ln_beta, dtype=np.float32)
    out = out * g[None, None, :] + be[None, None, :]
    return out.astype(np.float32)
